# revision 32
# baseline (speedup 1.0000x reference)
"""Trainium2 Bass kernel: grouped-pointwise FFN with channel shuffle.

Computes (per batch b, all ops pointwise in T):
    h   = W1_grouped @ (x * mask) + b1          # G=4 block-diagonal GEMM
    h   = channel_shuffle(h, G)
    h   = gelu(h)                               # exact erf gelu
    out = (W2_grouped @ h + b2) * mask

Sharding: data-parallel over batch B=16 across 8 cores (2 batches/core).
Weights are replicated; no collectives.

The spec pins x_mask to all-ones, so the kernel drops the mask path on
device; if a caller ever passes a non-trivial mask it is applied exactly
on the host (x*mask pre, out*mask post) which commutes with the kernel.

Engine budget per core (pace analysis):
  ACT (gelu, 1.2GHz, dtype-independent): 64 ops x [128,1024] ~ 65us  <- pace
  PE  (256 matmuls x 512 free, bf16 1cyc/row @2.4GHz): ~55-61us
  DVE (GEMM2 drain +bias): 32 x [128,512] ~ 23us
  DMA (bf16 in 5.2MB, f32 out 8.4MB): ~41us wire
ACT is the critical engine; the schedule keeps it streaming back-to-back
(measured <1.5us of gaps): GEMM1 halves feed a 3-buf [128,1024] PSUM
pool (6 banks), GEMM2 a 2-buf [128,512] pool (2 banks); GEMM2 of
iteration i interleaves with GEMM1 of iteration i+1 (lag 1).

Head: a dummy 8-element gelu pulls the 1.3us Gelu ACT_TABLE_LOAD off
ACT#1's critical path; first-need DMAs split across both DGE rings in
consumption order. Tail: the last iteration's GEMM2 c0/c1 run inside its
h1 phase, c2/c3 burst after the last gelu with the two drains on
ACT(Identity+bias) and DVE in parallel, stores on the sync HWDGE ring
(SWDGE costs ~1us extra gen latency). Measured ~86-88us end-to-end
(baseline fp32r version: 113us); head/tail DMA jitter is ~+/-1.5us.

Channel shuffle is free: GEMM2's weight blocks are pre-gathered on the
host so GEMM2 group g2 contracts directly over GEMM1's (g, m=g2) tiles.

All matmul operands are bf16 (same PE rate as fp32r; half the DMA bytes
and SBUF); PSUM stays fp32, gelu output h is bf16, final out fp32.
"""

import numpy as np

import concourse.mybir as mybir
import concourse.tile as tile
from concourse import bacc
from concourse import bass_utils

F32 = mybir.dt.float32
BF16 = mybir.dt.bfloat16

N_CORES = 8
B, CIN, T = 16, 512, 2048
H, COUT, G = 2048, 512, 4
BPC = B // N_CORES        # batches per core
MB = (H // G) // 128      # 4 output-channel blocks per group in GEMM1
CH = 512                  # matmul free dim (1 PSUM bank)
AW = 1024                 # ACT op width (2 PSUM banks)

MM_DT = BF16

_compiled = {}


def _build(mm_dt):
    nc = bacc.Bacc(
        "TRN2", target_bir_lowering=False, debug=False, num_devices=N_CORES
    )
    xs = nc.dram_tensor("xs", [BPC * G, 128, T], mm_dt, kind="ExternalInput").ap()
    # wpk cols: w1t [(m, g, o)-major, 2048] then w2t [(g2, g, o)-major, 2048]
    wpk = nc.dram_tensor("wpk", [128, 2 * G * MB * 128], mm_dt, kind="ExternalInput").ap()
    # bpk cols: b1t [m*G+g, 16] then b2t [g2, 4]
    bpk = nc.dram_tensor("bpk", [128, G * MB + G], F32, kind="ExternalInput").ap()
    outs = nc.dram_tensor("outs", [BPC * G, 128, T], F32, kind="ExternalOutput").ap()

    with tile.TileContext(nc) as tc:
        with (
            tc.tile_pool(name="consts", bufs=1) as cpool,
            tc.tile_pool(name="xp", bufs=BPC * G) as xpool,
            tc.tile_pool(name="hp", bufs=2 * G) as hpool,
            tc.tile_pool(name="op", bufs=2) as opool,
            tc.tile_pool(name="ps1p", bufs=3, space="PSUM") as ps1pool,
            tc.tile_pool(name="ps2p", bufs=2, space="PSUM") as ps2pool,
        ):
            w_sb = cpool.tile([128, 2 * G * MB * 128], mm_dt)
            b_sb = cpool.tile([128, G * MB + G], F32)
            wup = cpool.tile([1, 8], mm_dt)
            dmy = cpool.tile([1, 8], mm_dt)
            x_sb = [[None] * G for _ in range(BPC)]

            # dummy gelu pulls the Gelu ACT_TABLE_LOAD (1.3us) off the
            # first real activation's critical path.
            nc.gpsimd.memset(wup.bitcast(mybir.dt.uint16), 0x3F80)
            nc.scalar.activation(
                dmy, wup, mybir.ActivationFunctionType.Gelu, scale=1.0
            )

            # head DMAs: the two first-need chunks go out in parallel on
            # separate rings (w1 m=0 on sync, x(0,0) on gpsimd), then the
            # rest in use-order.
            nc.sync.dma_start(w_sb[:, 0 : G * 128], wpk[:, 0 : G * 128])
            xt00 = xpool.tile([128, T], mm_dt, tag="x", name="xt")
            nc.gpsimd.dma_start(xt00[:, 0:CH], xs[0][:, 0:CH])
            nc.sync.dma_start(b_sb, bpk)
            nc.gpsimd.dma_start(xt00[:, CH:T], xs[0][:, CH:T])
            x_sb[0][0] = xt00
            xt01 = xpool.tile([128, T], mm_dt, tag="x", name="xt01")
            nc.sync.dma_start(xt01, xs[1])
            x_sb[0][1] = xt01
            nc.sync.dma_start(
                w_sb[:, G * 128 : G * MB * 128], wpk[:, G * 128 : G * MB * 128]
            )
            for g in range(2, G):
                xtg = xpool.tile([128, T], mm_dt, tag="x", name="xtg")
                nc.sync.dma_start(xtg, xs[g])
                x_sb[0][g] = xtg
            nc.sync.dma_start(
                w_sb[:, G * MB * 128 :], wpk[:, G * MB * 128 :]
            )

            def load_x(b, g):
                xt = xpool.tile([128, T], mm_dt, tag="x", name="xt")
                nc.sync.dma_start(xt, xs[b * G + g])
                x_sb[b][g] = xt

            def g1_half(b, m, g, half, ht):
                # one [128,1024] PSUM tile: 2 matmuls + fused gelu/bias
                ps1 = ps1pool.tile([128, AW], F32, tag="ps1", name="ps1")
                w_ap = w_sb[:, (m * G + g) * 128 : (m * G + g + 1) * 128]
                for c2 in range(AW // CH):
                    c = half * (AW // CH) + c2
                    nc.tensor.matmul(
                        ps1[:, c2 * CH : (c2 + 1) * CH],
                        w_ap,
                        x_sb[b][g][:, c * CH : (c + 1) * CH],
                        start=True, stop=True,
                    )
                nc.scalar.activation(
                    ht[:, half * AW : (half + 1) * AW],
                    ps1,
                    mybir.ActivationFunctionType.Gelu,
                    bias=b_sb[:, m * G + g : m * G + g + 1],
                    scale=1.0,
                )

            def g2_chunk(b, g2, hts, ot, c, fin=False, drain="dve"):
                cs = slice(c * CH, (c + 1) * CH)
                ps2 = ps2pool.tile([128, CH], F32, tag="ps2", name="ps2")
                for g in range(G):
                    wo = G * MB * 128 + (g2 * G + g) * 128
                    nc.tensor.matmul(
                        ps2,
                        w_sb[:, wo : wo + 128],
                        hts[g][:, cs],
                        start=(g == 0), stop=(g == G - 1),
                    )
                # out = psum + b2 (per-partition scalar), normally on DVE;
                # the tail's c2 drains on the (by then idle) ACT engine so
                # the last two drains run in parallel. Identity shares
                # Gelu's table set - no ACT_TABLE_LOAD.
                if drain == "act":
                    nc.scalar.activation(
                        ot[:, cs],
                        ps2,
                        mybir.ActivationFunctionType.Identity,
                        bias=b_sb[:, G * MB + g2 : G * MB + g2 + 1],
                        scale=1.0,
                    )
                else:
                    nc.vector.tensor_scalar_add(
                        ot[:, cs],
                        ps2,
                        b_sb[:, G * MB + g2 : G * MB + g2 + 1],
                    )
                if fin:
                    # tail: per-chunk stores on the idle sync HWDGE ring
                    # (SWDGE adds ~1us gen latency - bad for the last store)
                    nc.sync.dma_start(outs[b * G + g2][:, cs], ot[:, cs])
                elif c % 2 == 1:  # steady state: store half-tiles
                    os_ = slice((c - 1) * CH, (c + 1) * CH)
                    nc.gpsimd.dma_start(outs[b * G + g2][:, os_], ot[:, os_])

            # pipeline: GEMM2 of iter i-1 interleaves with GEMM1 of iter i;
            # the last iteration interleaves its own GEMM2 (after the h
            # halves it needs) to shorten the tail.
            NIT = BPC * MB
            prev = None
            for it in range(NIT):
                b, m = divmod(it, MB)
                last = it == NIT - 1
                hts = [
                    hpool.tile([128, T], mm_dt, tag="h", name="ht")
                    for _ in range(G)
                ]
                if prev is not None:
                    pot = opool.tile([128, T], F32, tag="o", name="pot")
                if not last:
                    for g in range(G):
                        g1_half(b, m, g, 0, hts[g])
                        g1_half(b, m, g, 1, hts[g])
                        if prev is not None:
                            g2_chunk(prev[0], prev[1], prev[2], pot, g)
                else:
                    # h0 phase carries prev-iter GEMM2; own c0/c1 follow it
                    # (PE has slack there), c2/c3 burst after the h1 phase.
                    fot = opool.tile([128, T], F32, tag="o", name="fot")
                    for g in range(G):
                        g1_half(b, m, g, 0, hts[g])
                        if prev is not None:
                            g2_chunk(prev[0], prev[1], prev[2], pot, g)
                    g2_chunk(b, m, hts, fot, 0, fin=True)
                    for g in range(G):
                        g1_half(b, m, g, 1, hts[g])
                        if g == 0:
                            g2_chunk(b, m, hts, fot, 1, fin=True)
                # x prefetch for batch b+1 spread over early iterations
                if b + 1 < BPC and m in (1, 2):
                    for g in range(2):
                        load_x(b + 1, 2 * (m - 1) + g)
                prev = (b, m, hts)
            # tail: the last two GEMM2 chunks of the last iteration
            for c in (2, 3):
                g2_chunk(prev[0], prev[1], prev[2], fot, c, fin=True,
                         drain="act" if c == 2 else "dve")

    nc.compile()
    return nc


def get_nc(mm_dt=None):
    mm_dt = MM_DT if mm_dt is None else mm_dt
    if mm_dt not in _compiled:
        _compiled[mm_dt] = _build(mm_dt)
    return _compiled[mm_dt]


def prep_inputs(x, x_mask, w1, b1, w2, b2):
    """Host-side layout prep. Returns per-core in_maps."""
    import ml_dtypes

    bf16 = ml_dtypes.bfloat16
    x = np.asarray(x, dtype=np.float32)
    w1 = np.asarray(w1, dtype=np.float32)
    b1 = np.asarray(b1, dtype=np.float32)
    w2 = np.asarray(w2, dtype=np.float32)
    b2 = np.asarray(b2, dtype=np.float32)

    # w1 [H, CIN/G] -> lhsT blocks [i, (m, g, o)]
    w1r = w1.reshape(G, MB, 128, CIN // G)          # g, m, o, i
    w1t = np.transpose(w1r, (3, 1, 0, 2)).reshape(128, G * MB * 128)
    # w2 [COUT, H/G] -> lhsT blocks [r, (g2, g, o)]; GEMM2 group g2
    # contracts h tile (g, m=g2) row r against w2[g2*128+o, r*G+g]
    # (channel shuffle pre-applied).
    w2r = w2.reshape(G, 128, 128, G)                # g2, o, r, g
    w2t = np.transpose(w2r, (2, 0, 3, 1)).reshape(128, G * G * 128)
    wpk = np.ascontiguousarray(
        np.concatenate([w1t, w2t], axis=1).astype(bf16)
    )
    b1t = b1.reshape(G, MB, 128).transpose(2, 1, 0).reshape(128, G * MB)
    b2t = b2.reshape(G, 128).T
    bpk = np.ascontiguousarray(
        np.concatenate([b1t, b2t], axis=1).astype(np.float32)
    )

    xr = np.ascontiguousarray(
        x.reshape(N_CORES, BPC * G, 128, T).astype(bf16)
    )

    in_maps = []
    for k in range(N_CORES):
        in_maps.append({"xs": xr[k], "wpk": wpk, "bpk": bpk})
    return in_maps


def assemble_output(results):
    """results: list of 8 dicts with 'outs' [BPC*G, 128, T]."""
    parts = [r["outs"].reshape(BPC, G * 128, T) for r in results]
    return np.concatenate(parts, axis=0).astype(np.float32)


def kernel(x, x_mask, w1, b1, w2, b2, n_groups):
    assert int(n_groups) == G
    import os

    # NTFF tracing needs antenv.axon_hooks, absent on this image; make
    # sure an inherited BASS_TRACE can't push us onto that path.
    os.environ["BASS_NEVER_TRACE"] = "1"

    x = np.asarray(x, dtype=np.float32)
    x_mask = np.asarray(x_mask, dtype=np.float32)
    trivial_mask = bool(np.all(x_mask == 1.0))
    if not trivial_mask:
        # mask is per-(b,t): it commutes with the pointwise convs, so
        # exact host-side pre/post multiply preserves semantics.
        x = x * x_mask

    nc = get_nc()
    in_maps = prep_inputs(x, x_mask, w1, b1, w2, b2)
    res = bass_utils.run_bass_kernel_spmd(
        nc, in_maps, core_ids=list(range(N_CORES))
    )
    out = assemble_output(res.results)
    if not trivial_mask:
        out = out * x_mask
    return out


# revision 36
# speedup vs baseline: 1.0100x; 1.0100x over previous
"""Trainium2 Bass kernel: grouped-pointwise FFN with channel shuffle.

Computes (per batch b, all ops pointwise in T):
    h   = W1_grouped @ (x * mask) + b1          # G=4 block-diagonal GEMM
    h   = channel_shuffle(h, G)
    h   = gelu(h)                               # exact erf gelu
    out = (W2_grouped @ h + b2) * mask

Sharding: data-parallel over batch B=16 across 8 cores (2 batches/core).
Weights are replicated; no collectives.

The spec pins x_mask to all-ones, so the kernel drops the mask path on
device; if a caller ever passes a non-trivial mask it is applied exactly
on the host (x*mask pre, out*mask post) which commutes with the kernel.

Engine budget per core (pace analysis):
  ACT (gelu, 1.2GHz, dtype-independent): 64 ops x [128,1024] ~ 65us  <- pace
  PE  (256 matmuls x 512 free, bf16 1cyc/row @2.4GHz): ~55-61us
  DVE (GEMM2 drain +bias): 32 x [128,512] ~ 23us
  DMA (bf16 in 5.2MB, f32 out 8.4MB): ~41us wire
ACT is the critical engine; the schedule keeps it streaming back-to-back
(measured <1.5us of gaps): GEMM1 halves feed a 3-buf [128,1024] PSUM
pool (6 banks), GEMM2 a 2-buf [128,512] pool (2 banks); GEMM2 of
iteration i interleaves with GEMM1 of iteration i+1 (lag 1).

Head: a dummy 8-element gelu pulls the 1.3us Gelu ACT_TABLE_LOAD off
ACT#1's critical path; first-need DMAs split across both DGE rings in
consumption order. Tail: the last iteration's GEMM2 c0/c1 run inside its
h1 phase, c2/c3 burst after the last gelu with the two drains on
ACT(Identity+bias) and DVE in parallel, stores on the sync HWDGE ring
(SWDGE costs ~1us extra gen latency). Measured ~86-88us end-to-end
(baseline fp32r version: 113us); head/tail DMA jitter is ~+/-1.5us.

Channel shuffle is free: GEMM2's weight blocks are pre-gathered on the
host so GEMM2 group g2 contracts directly over GEMM1's (g, m=g2) tiles.

All matmul operands are bf16 (same PE rate as fp32r; half the DMA bytes
and SBUF); PSUM stays fp32, gelu output h is bf16, final out fp32.
"""

import numpy as np

import concourse.mybir as mybir
import concourse.tile as tile
from concourse import bacc
from concourse import bass_utils

F32 = mybir.dt.float32
BF16 = mybir.dt.bfloat16

N_CORES = 8
B, CIN, T = 16, 512, 2048
H, COUT, G = 2048, 512, 4
BPC = B // N_CORES        # batches per core
MB = (H // G) // 128      # 4 output-channel blocks per group in GEMM1
CH = 512                  # matmul free dim (1 PSUM bank)
AW = 1024                 # ACT op width (2 PSUM banks)

MM_DT = BF16

_compiled = {}


def _build(mm_dt):
    nc = bacc.Bacc(
        "TRN2", target_bir_lowering=False, debug=False, num_devices=N_CORES
    )
    xs = nc.dram_tensor("xs", [BPC * G, 128, T], mm_dt, kind="ExternalInput").ap()
    # wpk cols: w1t [(m, g, o)-major, 2048] then w2t [(g2, g, o)-major, 2048]
    wpk = nc.dram_tensor("wpk", [128, 2 * G * MB * 128], mm_dt, kind="ExternalInput").ap()
    # bpk cols: b1t [m*G+g, 16] then b2t [g2, 4]
    bpk = nc.dram_tensor("bpk", [128, G * MB + G], F32, kind="ExternalInput").ap()
    outs = nc.dram_tensor("outs", [BPC * G, 128, T], F32, kind="ExternalOutput").ap()

    with tile.TileContext(nc) as tc:
        with (
            tc.tile_pool(name="consts", bufs=1) as cpool,
            tc.tile_pool(name="xp", bufs=BPC * G) as xpool,
            tc.tile_pool(name="hp", bufs=2 * G) as hpool,
            tc.tile_pool(name="op", bufs=2) as opool,
            tc.tile_pool(name="ps1p", bufs=3, space="PSUM") as ps1pool,
            tc.tile_pool(name="ps2p", bufs=2, space="PSUM") as ps2pool,
        ):
            w_sb = cpool.tile([128, 2 * G * MB * 128], mm_dt)
            b_sb = cpool.tile([128, G * MB + G], F32)
            wup = cpool.tile([1, 8], mm_dt)
            dmy = cpool.tile([1, 8], mm_dt)
            x_sb = [[None] * G for _ in range(BPC)]

            # dummy gelu pulls the Gelu ACT_TABLE_LOAD (1.3us) off the
            # first real activation's critical path.
            nc.gpsimd.memset(wup.bitcast(mybir.dt.uint16), 0x3F80)
            nc.scalar.activation(
                dmy, wup, mybir.ActivationFunctionType.Gelu, scale=1.0
            )

            # head DMAs: the two first-need chunks go out in parallel on
            # separate rings (w1 m=0 on sync, x(0,0) on gpsimd), then the
            # rest in use-order.
            nc.sync.dma_start(w_sb[:, 0 : G * 128], wpk[:, 0 : G * 128])
            xt00 = xpool.tile([128, T], mm_dt, tag="x", name="xt")
            nc.gpsimd.dma_start(xt00[:, 0:CH], xs[0][:, 0:CH])
            nc.sync.dma_start(b_sb, bpk)
            nc.gpsimd.dma_start(xt00[:, CH:T], xs[0][:, CH:T])
            x_sb[0][0] = xt00
            xt01 = xpool.tile([128, T], mm_dt, tag="x", name="xt01")
            nc.sync.dma_start(xt01, xs[1])
            x_sb[0][1] = xt01
            nc.sync.dma_start(
                w_sb[:, G * 128 : G * MB * 128], wpk[:, G * 128 : G * MB * 128]
            )
            for g in range(2, G):
                xtg = xpool.tile([128, T], mm_dt, tag="x", name="xtg")
                nc.sync.dma_start(xtg, xs[g])
                x_sb[0][g] = xtg
            nc.sync.dma_start(
                w_sb[:, G * MB * 128 :], wpk[:, G * MB * 128 :]
            )

            def load_x(b, g):
                xt = xpool.tile([128, T], mm_dt, tag="x", name="xt")
                nc.sync.dma_start(xt, xs[b * G + g])
                x_sb[b][g] = xt

            def g1_half(b, m, g, half, ht, aw=AW):
                # one [128,1024] PSUM tile: 2 matmuls + fused gelu/bias.
                # aw=CH emits one gelu per matmul (for the tail, where the
                # first 512 unblocks the held c2 accumulation early).
                ps1 = ps1pool.tile([128, AW], F32, tag="ps1", name="ps1")
                w_ap = w_sb[:, (m * G + g) * 128 : (m * G + g + 1) * 128]
                bias = b_sb[:, m * G + g : m * G + g + 1]
                for c2 in range(AW // CH):
                    c = half * (AW // CH) + c2
                    nc.tensor.matmul(
                        ps1[:, c2 * CH : (c2 + 1) * CH],
                        w_ap,
                        x_sb[b][g][:, c * CH : (c + 1) * CH],
                        start=True, stop=True,
                    )
                    if aw == CH:
                        nc.scalar.activation(
                            ht[:, half * AW + c2 * CH : half * AW + (c2 + 1) * CH],
                            ps1[:, c2 * CH : (c2 + 1) * CH],
                            mybir.ActivationFunctionType.Gelu,
                            bias=bias,
                            scale=1.0,
                        )
                if aw == AW:
                    nc.scalar.activation(
                        ht[:, half * AW : (half + 1) * AW],
                        ps1,
                        mybir.ActivationFunctionType.Gelu,
                        bias=bias,
                        scale=1.0,
                    )

            def g2_chunk(b, g2, hts, ot, c, fin=False, drain="dve"):
                cs = slice(c * CH, (c + 1) * CH)
                ps2 = ps2pool.tile([128, CH], F32, tag="ps2", name="ps2")
                for g in range(G):
                    wo = G * MB * 128 + (g2 * G + g) * 128
                    nc.tensor.matmul(
                        ps2,
                        w_sb[:, wo : wo + 128],
                        hts[g][:, cs],
                        start=(g == 0), stop=(g == G - 1),
                    )
                # out = psum + b2 (per-partition scalar), normally on DVE;
                # the tail's c2 drains on the (by then idle) ACT engine so
                # the last two drains run in parallel. Identity shares
                # Gelu's table set - no ACT_TABLE_LOAD.
                if drain == "act":
                    nc.scalar.activation(
                        ot[:, cs],
                        ps2,
                        mybir.ActivationFunctionType.Identity,
                        bias=b_sb[:, G * MB + g2 : G * MB + g2 + 1],
                        scale=1.0,
                    )
                else:
                    nc.vector.tensor_scalar_add(
                        ot[:, cs],
                        ps2,
                        b_sb[:, G * MB + g2 : G * MB + g2 + 1],
                    )
                if fin:
                    # tail: per-chunk stores on the idle sync HWDGE ring
                    # (SWDGE adds ~1us gen latency - bad for the last store)
                    nc.sync.dma_start(outs[b * G + g2][:, cs], ot[:, cs])
                elif c % 2 == 1:  # steady state: store half-tiles
                    os_ = slice((c - 1) * CH, (c + 1) * CH)
                    nc.gpsimd.dma_start(outs[b * G + g2][:, os_], ot[:, os_])

            # pipeline: GEMM2 of iter i-1 interleaves with GEMM1 of iter i;
            # the last iteration interleaves its own GEMM2 (after the h
            # halves it needs) to shorten the tail.
            NIT = BPC * MB
            prev = None
            for it in range(NIT):
                b, m = divmod(it, MB)
                last = it == NIT - 1
                hts = [
                    hpool.tile([128, T], mm_dt, tag="h", name="ht")
                    for _ in range(G)
                ]
                if prev is not None:
                    pot = opool.tile([128, T], F32, tag="o", name="pot")
                if not last:
                    for g in range(G):
                        g1_half(b, m, g, 0, hts[g])
                        g1_half(b, m, g, 1, hts[g])
                        if prev is not None:
                            g2_chunk(prev[0], prev[1], prev[2], pot, g)
                else:
                    # h0 phase carries prev-iter GEMM2; own c0/c1 follow it
                    # (PE has slack there). c2 accumulates per-group through
                    # the h1 phase into a held psum tile, with the last gelu
                    # split 2x512 so c2 completes on its first 512; c3 bursts
                    # at the very end. ps2 slot order (2-buf rotation) keeps
                    # the held c2 tile and c3 off each other's drains.
                    fot = opool.tile([128, T], F32, tag="o", name="fot")
                    for g in range(G):
                        g1_half(b, m, g, 0, hts[g])
                        if prev is not None:
                            g2_chunk(prev[0], prev[1], prev[2], pot, g)
                    g2_chunk(b, m, hts, fot, 0, fin=True)
                    c2s = slice(2 * CH, 3 * CH)
                    ps2_c2 = None
                    for g in range(G):
                        g1_half(b, m, g, 1, hts[g], aw=CH if g == 3 else AW)
                        if g == 0:
                            g2_chunk(b, m, hts, fot, 1, fin=True)
                            ps2_c2 = ps2pool.tile(
                                [128, CH], F32, tag="ps2", name="c2h"
                            )
                        wo = G * MB * 128 + (m * G + g) * 128
                        nc.tensor.matmul(
                            ps2_c2, w_sb[:, wo : wo + 128],
                            hts[g][:, c2s], start=(g == 0), stop=(g == G - 1),
                        )
                    nc.vector.tensor_scalar_add(
                        fot[:, c2s], ps2_c2, b_sb[:, G * MB + m : G * MB + m + 1]
                    )
                    nc.sync.dma_start(outs[b * G + m][:, c2s], fot[:, c2s])
                # x prefetch for batch b+1 spread over early iterations
                if b + 1 < BPC and m in (1, 2):
                    for g in range(2):
                        load_x(b + 1, 2 * (m - 1) + g)
                prev = (b, m, hts)
            # tail: only c3 remains; drain on the now-idle ACT engine while
            # c2's store transfer is already in flight
            g2_chunk(prev[0], prev[1], prev[2], fot, 3, fin=True,
                     drain="act")

    nc.compile()
    return nc


def get_nc(mm_dt=None):
    mm_dt = MM_DT if mm_dt is None else mm_dt
    if mm_dt not in _compiled:
        _compiled[mm_dt] = _build(mm_dt)
    return _compiled[mm_dt]


def prep_inputs(x, x_mask, w1, b1, w2, b2):
    """Host-side layout prep. Returns per-core in_maps."""
    import ml_dtypes

    bf16 = ml_dtypes.bfloat16
    x = np.asarray(x, dtype=np.float32)
    w1 = np.asarray(w1, dtype=np.float32)
    b1 = np.asarray(b1, dtype=np.float32)
    w2 = np.asarray(w2, dtype=np.float32)
    b2 = np.asarray(b2, dtype=np.float32)

    # w1 [H, CIN/G] -> lhsT blocks [i, (m, g, o)]
    w1r = w1.reshape(G, MB, 128, CIN // G)          # g, m, o, i
    w1t = np.transpose(w1r, (3, 1, 0, 2)).reshape(128, G * MB * 128)
    # w2 [COUT, H/G] -> lhsT blocks [r, (g2, g, o)]; GEMM2 group g2
    # contracts h tile (g, m=g2) row r against w2[g2*128+o, r*G+g]
    # (channel shuffle pre-applied).
    w2r = w2.reshape(G, 128, 128, G)                # g2, o, r, g
    w2t = np.transpose(w2r, (2, 0, 3, 1)).reshape(128, G * G * 128)
    wpk = np.ascontiguousarray(
        np.concatenate([w1t, w2t], axis=1).astype(bf16)
    )
    b1t = b1.reshape(G, MB, 128).transpose(2, 1, 0).reshape(128, G * MB)
    b2t = b2.reshape(G, 128).T
    bpk = np.ascontiguousarray(
        np.concatenate([b1t, b2t], axis=1).astype(np.float32)
    )

    xr = np.ascontiguousarray(
        x.reshape(N_CORES, BPC * G, 128, T).astype(bf16)
    )

    in_maps = []
    for k in range(N_CORES):
        in_maps.append({"xs": xr[k], "wpk": wpk, "bpk": bpk})
    return in_maps


def assemble_output(results):
    """results: list of 8 dicts with 'outs' [BPC*G, 128, T]."""
    parts = [r["outs"].reshape(BPC, G * 128, T) for r in results]
    return np.concatenate(parts, axis=0).astype(np.float32)


def kernel(x, x_mask, w1, b1, w2, b2, n_groups):
    assert int(n_groups) == G
    import os

    # NTFF tracing needs antenv.axon_hooks, absent on this image; make
    # sure an inherited BASS_TRACE can't push us onto that path.
    os.environ["BASS_NEVER_TRACE"] = "1"

    x = np.asarray(x, dtype=np.float32)
    x_mask = np.asarray(x_mask, dtype=np.float32)
    trivial_mask = bool(np.all(x_mask == 1.0))
    if not trivial_mask:
        # mask is per-(b,t): it commutes with the pointwise convs, so
        # exact host-side pre/post multiply preserves semantics.
        x = x * x_mask

    nc = get_nc()
    in_maps = prep_inputs(x, x_mask, w1, b1, w2, b2)
    res = bass_utils.run_bass_kernel_spmd(
        nc, in_maps, core_ids=list(range(N_CORES))
    )
    out = assemble_output(res.results)
    if not trivial_mask:
        out = out * x_mask
    return out
